# revision 1
# baseline (speedup 1.0000x reference)
"""Trainium2 Bass kernel for ConvNdFunc: 16x16/stride-8 patch MLP (256->1024->1).

Data-parallel over batch: 32 images -> 8 cores x 4 images, no collectives.

Host pre-computes a dense im2col phase layout, one buffer per K-chunk c:
xd[b, g, p, c, i*63+j] = x[b, 8*(7g+i) + p//8, 8j + 8c + p%8] (p = kh*8+kwp),
so each 441-window tile (7 window-rows x 63 cols) loads with one contiguous
116KB DMA and both L1 chunk matmuls read flat stride-1 rhs APs. No on-chip
im2col, no padding waste.

Per tile (PE period ~3.86us, zero steady-state PE gaps):
  - L1 (TensorE): ht[hid128, 441] += W1_chunk.T @ patches, 8 hidden blocks x
    2 K-chunks = 16 bf16 matmuls (f32 PSUM accum, 6-bank pipeline).
  - ReLU PSUM -> bf16 SBUF: 5 blocks on ScalarE activation, 3 on VectorE max
    (order no longer matters: L2 is software-pipelined a tile behind).
  - L2 (TensorE): 8 accumulating M=1 matmuls as 4 concurrent column-strip
    pairs (tile_position (0,0)/(0,32)) at full 186ns/slot cadence, emitted
    two tiles at a time so the full-width->strip weight-load stall (~280ns)
    is paid once per two tiles; flushed unbatched near the kernel tail.
  - Merge on VectorE: stage strip-1 row to SBUF, then (strip0 + b2) + strip1
    fused; one direct y-DMA per tile on the sync ring (no accum-DMA SWDGE
    latency on the tail).
  - Head: sync ring carries data only; w1 is sliced across the scalar +
    gpsimd rings in first-use order so tile 0 only gates on its own data
    and hb0 (cold-DMA clock makes early transfers 4-8x slow); zero matmuls
    warm the PE clock ramp while the first loads are in flight.

Measured: ~159-161us HW exec (baseline 171.4us), rel err ~3.4e-3 (bf16 data
path, f32 accumulate). TensorE runs gapless at peak bf16 cadence in steady
state; remaining overhead is the fixed preamble/clock-ramp head and the
drain/barrier tail.
"""

import os
import sys
from contextlib import ExitStack

_RT = "/opt/trn_rl_repo"
if _RT not in sys.path:
    sys.path.insert(0, _RT)

import ml_dtypes
import numpy as np

def _ensure_ntff_hook():
    """Register the axon NTFF profiling hook if the image's antenv lacks it.

    Only matters when tracing (KERNEL_TRACE=1); no-op side effects otherwise.
    """
    import types

    try:
        import antenv.axon_hooks  # noqa: F401

        return
    except ImportError:
        pass
    try:
        import antenv
    except ImportError:
        return
    mod = types.ModuleType("antenv.axon_hooks")
    _state = {"hook": None}
    mod.set_axon_ntff_profile_hook = lambda h: _state.__setitem__("hook", h)
    mod.get_axon_ntff_profile_hook = lambda: _state["hook"]
    sys.modules["antenv.axon_hooks"] = mod
    antenv.axon_hooks = mod
    try:
        from trn_agent_boot.trn_boot import _ntff_profile_via_ctypes

        mod.set_axon_ntff_profile_hook(
            _ntff_profile_via_ctypes("/opt/axon/libaxon_pjrt.so")
        )
    except Exception:
        pass


_ensure_ntff_hook()

import concourse.bass as bass
import concourse.tile as tile
from concourse import bacc, mybir
from concourse.bass_utils import run_bass_kernel_spmd

B, H, W = 32, 512, 512
KK, S, HID = 16, 8, 1024
OH = OW = (H - KK) // S + 1  # 63
NCORES = 8
BPC = B // NCORES  # 4 images per core
G = 7  # window-rows per tile
NG = OH // G  # 9 tiles per image
NWP = G * OW  # 441 matmul free dim per tile (7 window-rows x 63 cols)
NHB = HID // 128  # 8 hidden blocks

BF16 = ml_dtypes.bfloat16
F32 = mybir.dt.float32
BF16_T = mybir.dt.bfloat16

LAST_RESULTS = None  # BassKernelResults of the most recent run (for test harness)

HB_ORDER = list(range(NHB))
SCAL_HB = {0, 1, 2, 3, 4}  # ReLU on ScalarE; rest on VectorE (L2 is pipelined
# one tile behind, so ReLU completion order no longer gates L2 pairs)
N_WARM = 8


def _build_nc(b2_val: float, b1_nonzero: bool):
    nc = bacc.Bacc(None, target_bir_lowering=False)

    # host dense phase layout per K-chunk c:
    # x[b, g, p, c, i*63+j] = img[8*(G*g+i) + p//8, 8j + 8c + p%8]
    x_d = nc.dram_tensor("x", [BPC, NG, 128, 2, NWP], BF16_T, kind="ExternalInput")
    w1_d = nc.dram_tensor("w1", [128, 2, HID], BF16_T, kind="ExternalInput")
    w2_d = nc.dram_tensor("w2", [128, NHB], BF16_T, kind="ExternalInput")
    b1_d = nc.dram_tensor("b1", [1, HID], BF16_T, kind="ExternalInput")
    y_d = nc.dram_tensor("y", [BPC, OH, OW], F32, kind="ExternalOutput")

    relu = mybir.ActivationFunctionType.Relu

    with tile.TileContext(nc) as tc, ExitStack() as ctx:
        consts = ctx.enter_context(tc.tile_pool(name="consts", bufs=1))
        xin_pool = ctx.enter_context(tc.tile_pool(name="xin", bufs=4))
        hs_pool = ctx.enter_context(tc.tile_pool(name="hs", bufs=4))
        osb_pool = ctx.enter_context(tc.tile_pool(name="osb", bufs=8))
        ht_pool = ctx.enter_context(tc.tile_pool(name="ht", bufs=6, space="PSUM"))
        ops_pool = ctx.enter_context(tc.tile_pool(name="ops", bufs=2, space="PSUM"))

        w1_sb = consts.tile([128, 2, HID], BF16_T)
        w2_sb = consts.tile([128, NHB], BF16_T)

        # PE runs at a cold clock for the first ~5us of activity; a few zero
        # matmuls start the HAM ramp while the first DMAs are in flight.
        warm_in = consts.tile([128, 512], BF16_T)
        nc.gpsimd.memset(warm_in, 0.0)
        warm_ps = ht_pool.tile([128, NWP], F32, tag="ht")
        for _ in range(N_WARM):
            nc.tensor.matmul(
                warm_ps, warm_in[:, 0:128], warm_in[:, 0:NWP], start=True, stop=True
            )
        if b1_nonzero:
            b1_sb = consts.tile([1, HID], BF16_T)
            nc.scalar.dma_start(out=b1_sb, in_=b1_d[:, :])
            ones_sb = consts.tile([1, NWP], BF16_T)
            nc.vector.memset(ones_sb, 1.0)

        first = True

        def emit_l2(prev):
            # L2 for a previous tile, software-pipelined one tile behind L1 so
            # the PE never waits on that tile's ReLUs. Even blocks -> PE col
            # strip 0 (psum partition 0), odd -> strip 1 (partition 32); the
            # two rhs streams run concurrently so 8 matmuls cost ~4 slots.
            b, g, hs = prev
            ops = ops_pool.tile([33, NWP], F32)
            for k in range(NHB // 2):
                nc.tensor.matmul(
                    ops[0:1, :],
                    w2_sb[:, 2 * k : 2 * k + 1],
                    hs[:, 2 * k, :],
                    start=(k == 0),
                    stop=(k == NHB // 2 - 1),
                    tile_position=(0, 0),
                )
                nc.tensor.matmul(
                    ops[32:33, :],
                    w2_sb[:, 2 * k + 1 : 2 * k + 2],
                    hs[:, 2 * k + 1, :],
                    start=(k == 0),
                    stop=(k == NHB // 2 - 1),
                    tile_position=(0, 32),
                )
            # merge on VectorE: stage strip-1 row to SBUF (DVE allows only
            # one PSUM operand per op), then (strip0 + b2) + strip1 fused; one
            # direct y-DMA on the sync ring -- no accum-DMA SWDGE latency.
            o32 = osb_pool.tile([1, NWP], F32, tag="o32")
            nc.vector.tensor_scalar_add(o32, ops[32:33, :], 0.0)
            osb = osb_pool.tile([1, NWP], F32)
            nc.vector.scalar_tensor_tensor(
                osb,
                ops[0:1, :],
                float(b2_val),
                o32,
                mybir.AluOpType.add,
                mybir.AluOpType.add,
            )
            out_src = bass.AP(
                tensor=osb.tensor,
                offset=osb.offset,
                ap=[osb.ap[0], [OW, G], [1, OW]],
            )
            nc.sync.dma_start(out=y_d[b, g * G : (g + 1) * G, :], in_=out_src)

        pend = []
        for b in range(BPC):
            for g in range(NG):
                xin = xin_pool.tile([128, 2, NWP], BF16_T)
                nc.sync.dma_start(out=xin, in_=x_d[b, g])
                if first:
                    # three concurrent cold-DMA streams so nothing serializes
                    # behind the 113KB tile-0 load (sync ring = data only):
                    # scalar ring carries the first two c1 slices ahead of its
                    # activation-table load; gpsimd the rest of w1 + w2,
                    # sliced in order of first use by tile 0's L1 slots.
                    for hb in range(2):
                        nc.scalar.dma_start(
                            out=w1_sb[:, 1, hb * 128 : (hb + 1) * 128],
                            in_=w1_d[:, 1, hb * 128 : (hb + 1) * 128],
                        )
                    for hb in range(2):
                        nc.gpsimd.dma_start(
                            out=w1_sb[:, 0, hb * 128 : (hb + 1) * 128],
                            in_=w1_d[:, 0, hb * 128 : (hb + 1) * 128],
                        )
                    for lo in range(256, HID, 256):
                        for c in range(2):
                            nc.gpsimd.dma_start(
                                out=w1_sb[:, c, lo : lo + 256],
                                in_=w1_d[:, c, lo : lo + 256],
                            )
                    nc.gpsimd.dma_start(out=w2_sb, in_=w2_d[:, :])
                first = False

                hs = hs_pool.tile([128, NHB, NWP], BF16_T)
                for hb in HB_ORDER:
                    ht = ht_pool.tile([128, NWP], F32)
                    if b1_nonzero:
                        nc.tensor.matmul(
                            ht[:, 0:NWP],
                            b1_sb[:, hb * 128 : (hb + 1) * 128],
                            ones_sb[:, :],
                            start=True,
                            stop=False,
                        )
                    for c in range(2):
                        nc.tensor.matmul(
                            ht[:, 0:NWP],
                            w1_sb[:, c, hb * 128 : (hb + 1) * 128],
                            xin[:, c, :],
                            start=(c == 0 and not b1_nonzero),
                            stop=(c == 1),
                        )
                    if hb in SCAL_HB:
                        nc.scalar.activation(
                            out=hs[:, hb, :], in_=ht[:, 0:NWP], func=relu
                        )
                    else:
                        nc.vector.tensor_scalar_max(
                            hs[:, hb, :], ht[:, 0:NWP], 0.0
                        )

                # batch the pipelined L2s two tiles at a time: one strip
                # transition (~280ns of PE drain-wait) per two tiles
                pend.append((b, g, hs))
                t = b * NG + g
                if t >= BPC * NG - 3:
                    # near the kernel tail: flush immediately so the final
                    # merges + output DMAs overlap the remaining compute
                    while pend:
                        emit_l2(pend.pop(0))
                elif len(pend) >= 2 and t % 2 == 0:
                    emit_l2(pend.pop(0))
                    emit_l2(pend.pop(0))
        for p_ in pend:
            emit_l2(p_)

    nc.finalize()
    return nc


def kernel(x, W1, b1, W2, b2):
    global LAST_RESULTS
    x = np.asarray(x, dtype=np.float32)
    W1 = np.asarray(W1, dtype=np.float32)
    b1 = np.asarray(b1, dtype=np.float32)
    W2 = np.asarray(W2, dtype=np.float32)
    b2 = np.asarray(b2, dtype=np.float32)

    xb = x.astype(BF16)
    # dense im2col phase layout (see module docstring): per (tile, partition,
    # K-chunk) 441 contiguous cols [i (7 window-rows) x j (63 cols)]
    xd = np.empty((B, NG, 128, 2, NWP), dtype=BF16)
    for p in range(128):
        kh, kwp = p // S, p % S
        for c in range(2):
            xs = xb[:, kh::S, kwp + S * c :: S][:, :OH, :OW]  # [B, 63, 63]
            xd[:, :, p, c, :] = xs.reshape(B, NG, NWP)

    # W1 row r = kh*16 + kw; chunk c, partition p=(kh*8+kwp) <- row kh*16 + 8*c + kwp
    w1p = (
        W1.reshape(KK, 2, S, HID).transpose(0, 2, 1, 3).reshape(128, 2, HID)
    ).astype(BF16)
    w2p = W2.reshape(NHB, 128).T.copy().astype(BF16)  # [p, hb] = W2[hb*128+p]
    b1p = b1.reshape(1, HID).astype(BF16)
    b1_nonzero = bool(np.any(b1 != 0.0))
    b2_val = float(b2.reshape(-1)[0])

    nc = _build_nc(b2_val, b1_nonzero)

    in_maps = []
    for c in range(NCORES):
        in_maps.append(
            {
                "x": np.ascontiguousarray(xd[c * BPC : (c + 1) * BPC]),
                "w1": w1p,
                "w2": w2p,
                "b1": b1p,
            }
        )

    LAST_RESULTS = run_bass_kernel_spmd(
        nc,
        in_maps,
        core_ids=list(range(NCORES)),
        trace=bool(int(os.environ.get("KERNEL_TRACE", "0") or "0")),
    )
    y = np.concatenate([r["y"] for r in LAST_RESULTS.results], axis=0)
    return y.astype(np.float32)



# revision 7
# speedup vs baseline: 1.0474x; 1.0474x over previous
"""Trainium2 Bass kernel for ConvNdFunc: 16x16/stride-8 patch MLP (256->1024->1).

Data-parallel over batch: 32 images -> 8 cores x 4 images, no collectives.

Host pre-computes a dense im2col phase layout, one buffer per K-chunk c:
xd[b, g, p, c, i*63+j] = x[b, 8*(7g+i) + p//8, 8j + 8c + p%8] (p = kh*8+kwp),
so each 441-window tile (7 window-rows x 63 cols) loads with one contiguous
116KB DMA and both L1 chunk matmuls read flat stride-1 rhs APs. No on-chip
im2col, no padding waste.

Per tile (PE period ~3.46us target, zero steady-state PE gaps):
  - L1 (TensorE): ht[hid128, 441] += W1_chunk.T @ patches, 8 hidden blocks x
    2 K-chunks = 16 bf16 matmuls (f32 PSUM accum, 6-bank pipeline).
  - ReLU PSUM -> bf16 SBUF: 5 blocks on ScalarE activation, 3 on VectorE max
    (order no longer matters: L2 is software-pipelined a tile behind).
  - L2 (TensorE): 8 accumulating M=1 matmuls on 4 concurrent column strips
    (tile_position (0,0)/(0,32)/(0,64)/(0,96)), 2 per strip -> ~2 slots per
    tile; emitted two tiles at a time so the full-width->strip weight-load
    stall is paid once per two tiles; flushed unbatched near the kernel tail.
  - Merge: DVE ops allow one PSUM operand and only 32-multiple partition
    shifts, so: u[0:33] = ops[64:97] + b2/2 (stage), v[0:33] = ops[0:33] + u
    (lanes 0/32 hold the two partials), then fold lanes with two accumulating
    gpsimd SWDGE DMAs onto the zero-initialized y block. The last N_TAIL
    tiles fold on VectorE + direct sync-ring DMA instead (no SWDGE latency
    on the kernel tail).
  - Head: tile-0's x load is split c0/c1 across the sync + scalar HWDGE
    rings; w1 sliced across gpsimd (hb0-3) + scalar (hb4-7) in first-use
    order so tile 0 only gates on its own slices (cold-DMA clock makes early
    transfers 4-8x slow); zero matmuls warm the PE clock ramp while the
    first loads are in flight.

Baseline session: ~160.5us. This version targets ~140us: L2 strip slots
halved (-13us), head stalls reduced. rel err ~3.4e-3 (bf16 data path, f32
accumulate).
"""

import os
import sys
from contextlib import ExitStack

_RT = "/opt/trn_rl_repo"
if _RT not in sys.path:
    sys.path.insert(0, _RT)

import ml_dtypes
import numpy as np

def _ensure_ntff_hook():
    """Register the axon NTFF profiling hook if the image's antenv lacks it.

    Only matters when tracing (KERNEL_TRACE=1); no-op side effects otherwise.
    """
    import types

    try:
        import antenv.axon_hooks  # noqa: F401

        return
    except ImportError:
        pass
    try:
        import antenv
    except ImportError:
        return
    mod = types.ModuleType("antenv.axon_hooks")
    _state = {"hook": None}
    mod.set_axon_ntff_profile_hook = lambda h: _state.__setitem__("hook", h)
    mod.get_axon_ntff_profile_hook = lambda: _state["hook"]
    sys.modules["antenv.axon_hooks"] = mod
    antenv.axon_hooks = mod
    try:
        from trn_agent_boot.trn_boot import _ntff_profile_via_ctypes

        mod.set_axon_ntff_profile_hook(
            _ntff_profile_via_ctypes("/opt/axon/libaxon_pjrt.so")
        )
    except Exception:
        pass


_ensure_ntff_hook()

import concourse.bass as bass
import concourse.tile as tile
from concourse import bacc, mybir
from concourse.bass_utils import run_bass_kernel_spmd

B, H, W = 32, 512, 512
KK, S, HID = 16, 8, 1024
OH = OW = (H - KK) // S + 1  # 63
NCORES = 8
BPC = B // NCORES  # 4 images per core
G = 7  # window-rows per tile
NG = OH // G  # 9 tiles per image
NWP = G * OW  # 441 matmul free dim per tile (7 window-rows x 63 cols)
NHB = HID // 128  # 8 hidden blocks

BF16 = ml_dtypes.bfloat16
F32 = mybir.dt.float32
BF16_T = mybir.dt.bfloat16

LAST_RESULTS = None  # BassKernelResults of the most recent run (for test harness)

HB_ORDER = list(range(NHB))
SCAL_HB = {0, 1, 2, 3, 4}  # ReLU on ScalarE; rest on VectorE (L2 is pipelined
# one tile behind, so ReLU completion order no longer gates L2 pairs)
N_WARM = 6
N_TAIL = 3  # last tiles use the direct (sync-DMA) merge to avoid SWDGE latency


def _build_nc(b2_val: float, b1_nonzero: bool):
    nc = bacc.Bacc(None, target_bir_lowering=False)

    # host dense phase layout per K-chunk c:
    # x[b, g, p, c, i*63+j] = img[8*(G*g+i) + p//8, 8j + 8c + p%8]
    x_d = nc.dram_tensor("x", [BPC, NG, 128, 2, NWP], BF16_T, kind="ExternalInput")
    w1_d = nc.dram_tensor("w1", [128, 2, HID], BF16_T, kind="ExternalInput")
    w2_d = nc.dram_tensor("w2", [128, NHB], BF16_T, kind="ExternalInput")
    b1_d = nc.dram_tensor("b1", [1, HID], BF16_T, kind="ExternalInput")
    y_d = nc.dram_tensor("y", [BPC, OH, OW], F32, kind="ExternalOutput")

    relu = mybir.ActivationFunctionType.Relu

    with tile.TileContext(nc) as tc, ExitStack() as ctx:
        consts = ctx.enter_context(tc.tile_pool(name="consts", bufs=1))
        xin_pool = ctx.enter_context(tc.tile_pool(name="xin", bufs=4))
        hs_pool = ctx.enter_context(tc.tile_pool(name="hs", bufs=4))
        osb_pool = ctx.enter_context(tc.tile_pool(name="osb", bufs=8))
        ht_pool = ctx.enter_context(tc.tile_pool(name="ht", bufs=6, space="PSUM"))
        ops_pool = ctx.enter_context(tc.tile_pool(name="ops", bufs=2, space="PSUM"))

        w1_sb = consts.tile([128, 2, HID], BF16_T)
        w2_sb = consts.tile([128, NHB], BF16_T)

        # PE runs at a cold clock for the first ~5us of activity; a few zero
        # matmuls start the HAM ramp while the first DMAs are in flight.
        warm_in = consts.tile([128, 512], BF16_T)
        nc.gpsimd.memset(warm_in, 0.0)
        warm_ps = ht_pool.tile([128, NWP], F32, tag="ht")
        for _ in range(N_WARM):
            nc.tensor.matmul(
                warm_ps, warm_in[:, 0:128], warm_in[:, 0:NWP], start=True, stop=True
            )
        if b1_nonzero:
            b1_sb = consts.tile([1, HID], BF16_T)
            nc.scalar.dma_start(out=b1_sb, in_=b1_d[:, :])
            ones_sb = consts.tile([1, NWP], BF16_T)
            nc.vector.memset(ones_sb, 1.0)

        first = True

        def emit_l2(prev, tail=False):
            # L2 for a previous tile, software-pipelined one tile behind L1 so
            # the PE never waits on that tile's ReLUs. Four concurrent column
            # strips (PE col groups 0/32/64/96, psum partitions likewise), two
            # accumulating matmuls per strip: 8 matmuls cost ~2 slots.
            b, g, hs = prev
            ops = ops_pool.tile([97, NWP], F32)
            for k in range(2):
                for s_i, part in enumerate((0, 32, 64, 96)):
                    hb = 4 * k + s_i
                    nc.tensor.matmul(
                        ops[part : part + 1, :],
                        w2_sb[:, hb : hb + 1],
                        hs[:, hb, :],
                        start=(k == 0),
                        stop=(k == 1),
                        tile_position=(0, part),
                    )
            # merge on VectorE (one PSUM operand per DVE op; partition shifts
            # must be multiples of 32): stage {64,96} down by 64 with +b2/2
            # each lane (b2 lands twice across the two lanes), then add slab
            # {0..32}. v lanes 0/32 hold the two partial sums.
            u = osb_pool.tile([33, NWP], F32, tag="u")
            nc.vector.tensor_scalar_add(u, ops[64:97, :], float(b2_val) * 0.5)
            v = osb_pool.tile([33, NWP], F32, tag="v")
            nc.vector.tensor_tensor(v, ops[0:33, :], u, mybir.AluOpType.add)
            if tail:
                # direct path: fold lanes on VectorE, one sync-ring y-DMA (no
                # accum-DMA SWDGE latency at the kernel tail)
                vs = osb_pool.tile([1, NWP], F32, tag="vs")
                nc.vector.tensor_scalar_add(vs, v[32:33, :], 0.0)
                osb = osb_pool.tile([1, NWP], F32)
                nc.vector.tensor_tensor(osb, v[0:1, :], vs, mybir.AluOpType.add)
                out_src = bass.AP(
                    tensor=osb.tensor,
                    offset=osb.offset,
                    ap=[osb.ap[0], [OW, G], [1, OW]],
                )
                nc.sync.dma_start(out=y_d[b, g * G : (g + 1) * G, :], in_=out_src)
            else:
                # fold lanes 0/32 with two accumulating SWDGE DMAs onto the
                # zero-initialized y block (same gpsimd queue -> ordered RMW)
                pstep = v.ap[0][0]
                for row in (0, 32):
                    src = bass.AP(
                        tensor=v.tensor,
                        offset=v.offset + row * pstep,
                        ap=[[pstep, 1], [OW, G], [1, OW]],
                    )
                    nc.gpsimd.dma_start(
                        out=y_d[b, g * G : (g + 1) * G, :],
                        in_=src,
                        accum_op=mybir.AluOpType.add,
                    )

        pend = []
        for b in range(BPC):
            for g in range(NG):
                xin = xin_pool.tile([128, 2, NWP], BF16_T)
                if first:
                    # split tile-0's 113KB load across both HWDGE rings (c0 on
                    # sync, c1 on scalar) so it lands ~2x sooner at the cold
                    # DMA clock; w1 goes on gpsimd in first-use order so hb k's
                    # matmuls only gate on their own slices, w2 last.
                    nc.sync.dma_start(out=xin[:, 0, :], in_=x_d[b, g, :, 0, :])
                    nc.scalar.dma_start(out=xin[:, 1, :], in_=x_d[b, g, :, 1, :])
                    nc.gpsimd.dma_start(
                        out=w1_sb[:, :, 0:128], in_=w1_d[:, :, 0:128]
                    )
                    for lo in range(128, 512, 128):
                        nc.gpsimd.dma_start(
                            out=w1_sb[:, :, lo : lo + 128],
                            in_=w1_d[:, :, lo : lo + 128],
                        )
                    for lo in range(512, HID, 256):
                        nc.scalar.dma_start(
                            out=w1_sb[:, :, lo : lo + 256],
                            in_=w1_d[:, :, lo : lo + 256],
                        )
                    nc.gpsimd.dma_start(out=w2_sb, in_=w2_d[:, :])
                else:
                    nc.sync.dma_start(out=xin, in_=x_d[b, g])
                first = False

                hs = hs_pool.tile([128, NHB, NWP], BF16_T)
                for hb in HB_ORDER:
                    ht = ht_pool.tile([128, NWP], F32)
                    if b1_nonzero:
                        nc.tensor.matmul(
                            ht[:, 0:NWP],
                            b1_sb[:, hb * 128 : (hb + 1) * 128],
                            ones_sb[:, :],
                            start=True,
                            stop=False,
                        )
                    for c in range(2):
                        nc.tensor.matmul(
                            ht[:, 0:NWP],
                            w1_sb[:, c, hb * 128 : (hb + 1) * 128],
                            xin[:, c, :],
                            start=(c == 0 and not b1_nonzero),
                            stop=(c == 1),
                        )
                    if hb in SCAL_HB:
                        nc.scalar.activation(
                            out=hs[:, hb, :], in_=ht[:, 0:NWP], func=relu
                        )
                    else:
                        nc.vector.tensor_scalar_max(
                            hs[:, hb, :], ht[:, 0:NWP], 0.0
                        )

                # batch the pipelined L2s two tiles at a time: one strip
                # transition (~280ns of PE drain-wait) per two tiles
                pend.append((b, g, hs))
                t = b * NG + g
                if t >= BPC * NG - 3:
                    # near the kernel tail: flush immediately so the final
                    # merges + output DMAs overlap the remaining compute
                    while pend:
                        p_ = pend.pop(0)
                        tl = p_[0] * NG + p_[1] >= BPC * NG - N_TAIL
                        emit_l2(p_, tail=tl)
                elif len(pend) >= 2 and t % 2 == 0:
                    emit_l2(pend.pop(0))
                    emit_l2(pend.pop(0))
        for p_ in pend:
            emit_l2(p_, tail=True)

    nc.finalize()
    return nc


def kernel(x, W1, b1, W2, b2):
    global LAST_RESULTS
    x = np.asarray(x, dtype=np.float32)
    W1 = np.asarray(W1, dtype=np.float32)
    b1 = np.asarray(b1, dtype=np.float32)
    W2 = np.asarray(W2, dtype=np.float32)
    b2 = np.asarray(b2, dtype=np.float32)

    xb = x.astype(BF16)
    # dense im2col phase layout (see module docstring): per (tile, partition,
    # K-chunk) 441 contiguous cols [i (7 window-rows) x j (63 cols)]
    xd = np.empty((B, NG, 128, 2, NWP), dtype=BF16)
    for p in range(128):
        kh, kwp = p // S, p % S
        for c in range(2):
            xs = xb[:, kh::S, kwp + S * c :: S][:, :OH, :OW]  # [B, 63, 63]
            xd[:, :, p, c, :] = xs.reshape(B, NG, NWP)

    # W1 row r = kh*16 + kw; chunk c, partition p=(kh*8+kwp) <- row kh*16 + 8*c + kwp
    w1p = (
        W1.reshape(KK, 2, S, HID).transpose(0, 2, 1, 3).reshape(128, 2, HID)
    ).astype(BF16)
    w2p = W2.reshape(NHB, 128).T.copy().astype(BF16)  # [p, hb] = W2[hb*128+p]
    b1p = b1.reshape(1, HID).astype(BF16)
    b1_nonzero = bool(np.any(b1 != 0.0))
    b2_val = float(b2.reshape(-1)[0])

    nc = _build_nc(b2_val, b1_nonzero)

    in_maps = []
    for c in range(NCORES):
        in_maps.append(
            {
                "x": np.ascontiguousarray(xd[c * BPC : (c + 1) * BPC]),
                "w1": w1p,
                "w2": w2p,
                "b1": b1p,
            }
        )

    LAST_RESULTS = run_bass_kernel_spmd(
        nc,
        in_maps,
        core_ids=list(range(NCORES)),
        trace=bool(int(os.environ.get("KERNEL_TRACE", "0") or "0")),
    )
    y = np.concatenate([r["y"] for r in LAST_RESULTS.results], axis=0)
    return y.astype(np.float32)



# revision 12
# speedup vs baseline: 1.0480x; 1.0006x over previous
"""Trainium2 Bass kernel for ConvNdFunc: 16x16/stride-8 patch MLP (256->1024->1).

Data-parallel over batch: 32 images -> 8 cores x 4 images, no collectives.

Host pre-computes a dense im2col phase layout, one buffer per K-chunk c:
xd[b, g, p, c, i*63+j] = x[b, 8*(7g+i) + p//8, 8j + 8c + p%8] (p = kh*8+kwp),
so each 441-window tile (7 window-rows x 63 cols) loads with one contiguous
116KB DMA and both L1 chunk matmuls read flat stride-1 rhs APs. No on-chip
im2col, no padding waste.

Per tile (PE period ~3.46us target, zero steady-state PE gaps):
  - L1 (TensorE): ht[hid128, 441] += W1_chunk.T @ patches, 8 hidden blocks x
    2 K-chunks = 16 bf16 matmuls (f32 PSUM accum, 6-bank pipeline).
  - ReLU PSUM -> bf16 SBUF: 5 blocks on ScalarE activation, 3 on VectorE max
    (order no longer matters: L2 is software-pipelined a tile behind).
  - L2 (TensorE): 8 accumulating M=1 matmuls on 4 concurrent column strips
    (tile_position (0,0)/(0,32)/(0,64)/(0,96)), 2 per strip -> ~2 slots per
    tile; emitted two tiles at a time so the full-width->strip weight-load
    stall is paid once per two tiles; flushed unbatched near the kernel tail.
  - Merge: DVE ops allow one PSUM operand and only 32-multiple partition
    shifts, so: u[0:33] = ops[64:97] + b2/2 (stage), v[0:33] = ops[0:33] + u
    (lanes 0/32 hold the two partials), then fold lanes with two accumulating
    gpsimd SWDGE DMAs onto the zero-initialized y block. The last N_TAIL
    tiles fold on VectorE + direct sync-ring DMA instead (no SWDGE latency
    on the kernel tail).
  - Head: tile-0's x load is split c0/c1 across the sync + scalar HWDGE
    rings; w1 sliced across gpsimd (hb0-3) + scalar (hb4-7) in first-use
    order so tile 0 only gates on its own slices (cold-DMA clock makes early
    transfers 4-8x slow); zero matmuls warm the PE clock ramp while the
    first loads are in flight.

Baseline session: ~160.5us. This version targets ~140us: L2 strip slots
halved (-13us), head stalls reduced. rel err ~3.4e-3 (bf16 data path, f32
accumulate).
"""

import os
import sys
from contextlib import ExitStack

_RT = "/opt/trn_rl_repo"
if _RT not in sys.path:
    sys.path.insert(0, _RT)

import ml_dtypes
import numpy as np

def _ensure_ntff_hook():
    """Register the axon NTFF profiling hook if the image's antenv lacks it.

    Only matters when tracing (KERNEL_TRACE=1); no-op side effects otherwise.
    """
    import types

    try:
        import antenv.axon_hooks  # noqa: F401

        return
    except ImportError:
        pass
    try:
        import antenv
    except ImportError:
        return
    mod = types.ModuleType("antenv.axon_hooks")
    _state = {"hook": None}
    mod.set_axon_ntff_profile_hook = lambda h: _state.__setitem__("hook", h)
    mod.get_axon_ntff_profile_hook = lambda: _state["hook"]
    sys.modules["antenv.axon_hooks"] = mod
    antenv.axon_hooks = mod
    try:
        from trn_agent_boot.trn_boot import _ntff_profile_via_ctypes

        mod.set_axon_ntff_profile_hook(
            _ntff_profile_via_ctypes("/opt/axon/libaxon_pjrt.so")
        )
    except Exception:
        pass


_ensure_ntff_hook()

import concourse.bass as bass
import concourse.tile as tile
from concourse import bacc, mybir
from concourse.bass_utils import run_bass_kernel_spmd

B, H, W = 32, 512, 512
KK, S, HID = 16, 8, 1024
OH = OW = (H - KK) // S + 1  # 63
NCORES = 8
BPC = B // NCORES  # 4 images per core
G = 7  # window-rows per tile
NG = OH // G  # 9 tiles per image
NWP = G * OW  # 441 matmul free dim per tile (7 window-rows x 63 cols)
NHB = HID // 128  # 8 hidden blocks

BF16 = ml_dtypes.bfloat16
F32 = mybir.dt.float32
BF16_T = mybir.dt.bfloat16

LAST_RESULTS = None  # BassKernelResults of the most recent run (for test harness)

HB_ORDER = list(range(NHB))
SCAL_HB = {0, 1, 2, 3, 4}  # ReLU on ScalarE; rest on VectorE (L2 is pipelined
# one tile behind, so ReLU completion order no longer gates L2 pairs)
N_WARM = 12  # ~267ns each (320-col): bridge PE from preamble to tile-0 data
N_WARM_COLS = 320
N_TAIL = 3  # last tiles use the direct (sync-DMA) merge to avoid SWDGE latency


def _build_nc(b2_val: float, b1_nonzero: bool):
    nc = bacc.Bacc(None, target_bir_lowering=False)

    # host dense phase layout per K-chunk c:
    # x[b, g, p, c, i*63+j] = img[8*(G*g+i) + p//8, 8j + 8c + p%8]
    x_d = nc.dram_tensor("x", [BPC, NG, 128, 2, NWP], BF16_T, kind="ExternalInput")
    w1_d = nc.dram_tensor("w1", [128, 2, HID], BF16_T, kind="ExternalInput")
    w2_d = nc.dram_tensor("w2", [128, NHB], BF16_T, kind="ExternalInput")
    b1_d = nc.dram_tensor("b1", [1, HID], BF16_T, kind="ExternalInput")
    y_d = nc.dram_tensor("y", [BPC, OH, OW], F32, kind="ExternalOutput")

    relu = mybir.ActivationFunctionType.Relu

    with tile.TileContext(nc) as tc, ExitStack() as ctx:
        consts = ctx.enter_context(tc.tile_pool(name="consts", bufs=1))
        xin_pool = ctx.enter_context(tc.tile_pool(name="xin", bufs=4))
        hs_pool = ctx.enter_context(tc.tile_pool(name="hs", bufs=4))
        osb_pool = ctx.enter_context(tc.tile_pool(name="osb", bufs=8))
        ht_pool = ctx.enter_context(tc.tile_pool(name="ht", bufs=6, space="PSUM"))
        ops_pool = ctx.enter_context(tc.tile_pool(name="ops", bufs=2, space="PSUM"))

        w1_sb = consts.tile([128, 2, HID], BF16_T)
        w2_sb = consts.tile([128, NHB], BF16_T)

        # PE runs at a cold clock for the first ~5us of activity; a few zero
        # matmuls start the HAM ramp while the first DMAs are in flight.
        warm_in = consts.tile([128, 512], BF16_T)
        nc.gpsimd.memset(warm_in, 0.0)
        warm_ps = ht_pool.tile([128, NWP], F32, tag="ht")
        for _ in range(N_WARM):
            nc.tensor.matmul(
                warm_ps[:, 0:N_WARM_COLS],
                warm_in[:, 0:128],
                warm_in[:, 0:N_WARM_COLS],
                start=True,
                stop=True,
            )
        if b1_nonzero:
            b1_sb = consts.tile([1, HID], BF16_T)
            nc.scalar.dma_start(out=b1_sb, in_=b1_d[:, :])
            ones_sb = consts.tile([1, NWP], BF16_T)
            nc.vector.memset(ones_sb, 1.0)

        first = True

        def emit_l2_mms(prev, v2, col):
            # L2 strip matmuls for one tile + the 2-op DVE merge into column
            # `col` of the shared pair buffer v2 [33, 2, NWP]. Four concurrent
            # column strips (PE col groups 0/32/64/96, psum partitions
            # likewise), two accumulating matmuls per strip: ~2 slots.
            b, g, hs = prev
            ops = ops_pool.tile([97, NWP], F32)
            for k in range(2):
                for s_i, part in enumerate((0, 32, 64, 96)):
                    hb = 4 * k + s_i
                    nc.tensor.matmul(
                        ops[part : part + 1, :],
                        w2_sb[:, hb : hb + 1],
                        hs[:, hb, :],
                        start=(k == 0),
                        stop=(k == 1),
                        tile_position=(0, part),
                    )
            # merge on VectorE (one PSUM operand per DVE op; partition shifts
            # must be multiples of 32): stage {64,96} down by 64 with +b2/2
            # each lane (b2 lands twice across the two lanes), then add slab
            # {0..32}. v2[:, col] lanes 0/32 hold the two partial sums.
            u = osb_pool.tile([33, NWP], F32, tag="u")
            nc.vector.tensor_scalar_add(u, ops[64:97, :], float(b2_val) * 0.5)
            nc.vector.tensor_tensor(
                v2[:, col, :], ops[0:33, :], u, mybir.AluOpType.add
            )

        def emit_l2_pair(prevA, prevB):
            # two consecutive tiles' y blocks are contiguous in DRAM (within
            # an image, and across the b/b+1 seam), so each lane row of the
            # shared v2 folds with ONE accumulating SWDGE DMA per lane onto
            # the zero-initialized 2-tile y span (same gpsimd queue -> ordered
            # read-modify-write).
            bA, gA, _ = prevA
            v2 = osb_pool.tile([33, 2, NWP], F32, tag="v2")
            emit_l2_mms(prevA, v2, 0)
            emit_l2_mms(prevB, v2, 1)
            pstep = v2.ap[0][0]
            y_flat = y_d.reshape([BPC * OH * OW])
            off = (bA * NG + gA) * NWP
            for row in (0, 32):
                src = bass.AP(
                    tensor=v2.tensor,
                    offset=v2.offset + row * pstep,
                    ap=[[pstep, 1], [NWP, 2], [1, NWP]],
                )
                nc.gpsimd.dma_start(
                    out=y_flat[off : off + 2 * NWP],
                    in_=src,
                    accum_op=mybir.AluOpType.add,
                )

        def emit_l2(prev, tail=False):
            # single-tile fallback (tail and odd flush tiles)
            b, g, hs = prev
            v2 = osb_pool.tile([33, 2, NWP], F32, tag="v2")
            emit_l2_mms(prev, v2, 0)
            pstep = v2.ap[0][0]
            if tail:
                # direct path: fold lanes on VectorE, one sync-ring y-DMA (no
                # accum-DMA SWDGE latency at the kernel tail)
                vs = osb_pool.tile([1, NWP], F32, tag="vs")
                nc.vector.tensor_scalar_add(vs, v2[32:33, 0, :], 0.0)
                osb = osb_pool.tile([1, NWP], F32)
                nc.vector.tensor_tensor(
                    osb, v2[0:1, 0, :], vs, mybir.AluOpType.add
                )
                out_src = bass.AP(
                    tensor=osb.tensor,
                    offset=osb.offset,
                    ap=[osb.ap[0], [OW, G], [1, OW]],
                )
                nc.sync.dma_start(out=y_d[b, g * G : (g + 1) * G, :], in_=out_src)
            else:
                for row in (0, 32):
                    src = bass.AP(
                        tensor=v2.tensor,
                        offset=v2.offset + row * pstep,
                        ap=[[pstep, 1], [OW, G], [1, OW]],
                    )
                    nc.gpsimd.dma_start(
                        out=y_d[b, g * G : (g + 1) * G, :],
                        in_=src,
                        accum_op=mybir.AluOpType.add,
                    )

        pend = []
        for b in range(BPC):
            for g in range(NG):
                xin = xin_pool.tile([128, 2, NWP], BF16_T)
                if first:
                    # split tile-0's 113KB load across both HWDGE rings (c0 on
                    # sync, c1 on scalar) so it lands ~2x sooner at the cold
                    # DMA clock; w1 goes on gpsimd in first-use order so hb k's
                    # matmuls only gate on their own slices, w2 last.
                    nc.sync.dma_start(out=xin[:, 0, :], in_=x_d[b, g, :, 0, :])
                    nc.scalar.dma_start(out=xin[:, 1, :], in_=x_d[b, g, :, 1, :])
                    nc.gpsimd.dma_start(
                        out=w1_sb[:, :, 0:128], in_=w1_d[:, :, 0:128]
                    )
                    for lo in range(128, 512, 128):
                        nc.gpsimd.dma_start(
                            out=w1_sb[:, :, lo : lo + 128],
                            in_=w1_d[:, :, lo : lo + 128],
                        )
                    for lo in range(512, HID, 256):
                        nc.scalar.dma_start(
                            out=w1_sb[:, :, lo : lo + 256],
                            in_=w1_d[:, :, lo : lo + 256],
                        )
                    nc.gpsimd.dma_start(out=w2_sb, in_=w2_d[:, :])
                else:
                    nc.sync.dma_start(out=xin, in_=x_d[b, g])
                first = False

                hs = hs_pool.tile([128, NHB, NWP], BF16_T)
                for hb in HB_ORDER:
                    ht = ht_pool.tile([128, NWP], F32)
                    if b1_nonzero:
                        nc.tensor.matmul(
                            ht[:, 0:NWP],
                            b1_sb[:, hb * 128 : (hb + 1) * 128],
                            ones_sb[:, :],
                            start=True,
                            stop=False,
                        )
                    for c in range(2):
                        nc.tensor.matmul(
                            ht[:, 0:NWP],
                            w1_sb[:, c, hb * 128 : (hb + 1) * 128],
                            xin[:, c, :],
                            start=(c == 0 and not b1_nonzero),
                            stop=(c == 1),
                        )
                    if hb in SCAL_HB:
                        nc.scalar.activation(
                            out=hs[:, hb, :], in_=ht[:, 0:NWP], func=relu
                        )
                    else:
                        nc.vector.tensor_scalar_max(
                            hs[:, hb, :], ht[:, 0:NWP], 0.0
                        )

                # batch the pipelined L2s two tiles at a time: one strip
                # transition (~280ns of PE drain-wait) per two tiles
                pend.append((b, g, hs))
                t = b * NG + g
                if t >= BPC * NG - 3:
                    # near the kernel tail: flush immediately so the final
                    # merges + output DMAs overlap the remaining compute
                    while pend:
                        p_ = pend.pop(0)
                        tl = p_[0] * NG + p_[1] >= BPC * NG - N_TAIL
                        emit_l2(p_, tail=tl)
                elif len(pend) >= 2 and t % 2 == 0:
                    emit_l2_pair(pend.pop(0), pend.pop(0))
        for p_ in pend:
            emit_l2(p_, tail=True)

    nc.finalize()
    return nc


def kernel(x, W1, b1, W2, b2):
    global LAST_RESULTS
    x = np.asarray(x, dtype=np.float32)
    W1 = np.asarray(W1, dtype=np.float32)
    b1 = np.asarray(b1, dtype=np.float32)
    W2 = np.asarray(W2, dtype=np.float32)
    b2 = np.asarray(b2, dtype=np.float32)

    xb = x.astype(BF16)
    # dense im2col phase layout (see module docstring): per (tile, partition,
    # K-chunk) 441 contiguous cols [i (7 window-rows) x j (63 cols)]
    xd = np.empty((B, NG, 128, 2, NWP), dtype=BF16)
    for p in range(128):
        kh, kwp = p // S, p % S
        for c in range(2):
            xs = xb[:, kh::S, kwp + S * c :: S][:, :OH, :OW]  # [B, 63, 63]
            xd[:, :, p, c, :] = xs.reshape(B, NG, NWP)

    # W1 row r = kh*16 + kw; chunk c, partition p=(kh*8+kwp) <- row kh*16 + 8*c + kwp
    w1p = (
        W1.reshape(KK, 2, S, HID).transpose(0, 2, 1, 3).reshape(128, 2, HID)
    ).astype(BF16)
    w2p = W2.reshape(NHB, 128).T.copy().astype(BF16)  # [p, hb] = W2[hb*128+p]
    b1p = b1.reshape(1, HID).astype(BF16)
    b1_nonzero = bool(np.any(b1 != 0.0))
    b2_val = float(b2.reshape(-1)[0])

    nc = _build_nc(b2_val, b1_nonzero)

    in_maps = []
    for c in range(NCORES):
        in_maps.append(
            {
                "x": np.ascontiguousarray(xd[c * BPC : (c + 1) * BPC]),
                "w1": w1p,
                "w2": w2p,
                "b1": b1p,
            }
        )

    LAST_RESULTS = run_bass_kernel_spmd(
        nc,
        in_maps,
        core_ids=list(range(NCORES)),
        trace=bool(int(os.environ.get("KERNEL_TRACE", "0") or "0")),
    )
    y = np.concatenate([r["y"] for r in LAST_RESULTS.results], axis=0)
    return y.astype(np.float32)



# revision 14
# speedup vs baseline: 1.0696x; 1.0206x over previous
"""Trainium2 Bass kernel for ConvNdFunc: 16x16/stride-8 patch MLP (256->1024->1).

Data-parallel over batch: 32 images -> 8 cores x 4 images, no collectives.

Host pre-computes a dense im2col phase layout, one buffer per K-chunk c:
xd[b, g, p, c, i*63+j] = x[b, 8*(7g+i) + p//8, 8j + 8c + p%8] (p = kh*8+kwp),
so each 441-window tile (7 window-rows x 63 cols) loads with one contiguous
116KB DMA and both L1 chunk matmuls read flat stride-1 rhs APs. No on-chip
im2col, no padding waste.

Per tile (PE period ~3.46us target, zero steady-state PE gaps):
  - L1 (TensorE): ht[hid128, 441] += W1_chunk.T @ patches, 8 hidden blocks x
    2 K-chunks = 16 bf16 matmuls (f32 PSUM accum, 6-bank pipeline).
  - ReLU PSUM -> bf16 SBUF: 5 blocks on ScalarE activation, 3 on VectorE max
    (order no longer matters: L2 is software-pipelined a tile behind).
  - L2 (TensorE): 8 accumulating M=1 matmuls on 4 concurrent column strips
    (tile_position (0,0)/(0,32)/(0,64)/(0,96)), 2 per strip -> ~2 slots per
    tile; emitted two tiles at a time so the full-width->strip weight-load
    stall is paid once per two tiles; flushed unbatched near the kernel tail.
  - Merge: DVE ops allow one PSUM operand and only 32-multiple partition
    shifts, so: u[0:33] = ops[64:97] + b2/2 (stage), v[0:33] = ops[0:33] + u
    (lanes 0/32 hold the two partials), then fold lanes with two accumulating
    gpsimd SWDGE DMAs onto the zero-initialized y block. The last N_TAIL
    tiles fold on VectorE + direct sync-ring DMA instead (no SWDGE latency
    on the kernel tail).
  - Head: tile-0's x load is split c0/c1 across the sync + scalar HWDGE
    rings; w1 sliced across gpsimd (hb0-3) + scalar (hb4-7) in first-use
    order so tile 0 only gates on its own slices (cold-DMA clock makes early
    transfers 4-8x slow); zero matmuls warm the PE clock ramp while the
    first loads are in flight.

Baseline session: ~160.5us. This version targets ~140us: L2 strip slots
halved (-13us), head stalls reduced. rel err ~3.4e-3 (bf16 data path, f32
accumulate).
"""

import os
import sys
from contextlib import ExitStack

_RT = "/opt/trn_rl_repo"
if _RT not in sys.path:
    sys.path.insert(0, _RT)

import ml_dtypes
import numpy as np

def _ensure_ntff_hook():
    """Register the axon NTFF profiling hook if the image's antenv lacks it.

    Only matters when tracing (KERNEL_TRACE=1); no-op side effects otherwise.
    """
    import types

    try:
        import antenv.axon_hooks  # noqa: F401

        return
    except ImportError:
        pass
    try:
        import antenv
    except ImportError:
        return
    mod = types.ModuleType("antenv.axon_hooks")
    _state = {"hook": None}
    mod.set_axon_ntff_profile_hook = lambda h: _state.__setitem__("hook", h)
    mod.get_axon_ntff_profile_hook = lambda: _state["hook"]
    sys.modules["antenv.axon_hooks"] = mod
    antenv.axon_hooks = mod
    try:
        from trn_agent_boot.trn_boot import _ntff_profile_via_ctypes

        mod.set_axon_ntff_profile_hook(
            _ntff_profile_via_ctypes("/opt/axon/libaxon_pjrt.so")
        )
    except Exception:
        pass


_ensure_ntff_hook()

import concourse.bass as bass
import concourse.tile as tile
from concourse import bacc, mybir
from concourse.bass_utils import run_bass_kernel_spmd

B, H, W = 32, 512, 512
KK, S, HID = 16, 8, 1024
OH = OW = (H - KK) // S + 1  # 63
NCORES = 8
BPC = B // NCORES  # 4 images per core
G = 7  # window-rows per tile
NG = OH // G  # 9 tiles per image
NWP = G * OW  # 441 matmul free dim per tile (7 window-rows x 63 cols)
NHB = HID // 128  # 8 hidden blocks

BF16 = ml_dtypes.bfloat16
F32 = mybir.dt.float32
BF16_T = mybir.dt.bfloat16

LAST_RESULTS = None  # BassKernelResults of the most recent run (for test harness)

HB_ORDER = list(range(NHB))
SCAL_HB = {0, 1, 2, 3, 4}  # ReLU on ScalarE; rest on VectorE (L2 is pipelined
# one tile behind, so ReLU completion order no longer gates L2 pairs)
N_WARM = 8  # ~267ns each (320-col): bridge PE from preamble to tile-0 data
N_WARM_COLS = 320
N_TAIL = 3  # last tiles use the direct (sync-DMA) merge to avoid SWDGE latency


def _build_nc(b2_val: float, b1_nonzero: bool):
    nc = bacc.Bacc(None, target_bir_lowering=False)

    # host dense phase layout per K-chunk c:
    # x[b, g, p, c, i*63+j] = img[8*(G*g+i) + p//8, 8j + 8c + p%8]
    x_d = nc.dram_tensor("x", [BPC, NG, 128, 2, NWP], BF16_T, kind="ExternalInput")
    w1_d = nc.dram_tensor("w1", [128, 2, HID], BF16_T, kind="ExternalInput")
    w2_d = nc.dram_tensor("w2", [128, NHB], BF16_T, kind="ExternalInput")
    b1_d = nc.dram_tensor("b1", [1, HID], BF16_T, kind="ExternalInput")
    y_d = nc.dram_tensor("y", [BPC, OH, OW], F32, kind="ExternalOutput")

    relu = mybir.ActivationFunctionType.Relu

    with tile.TileContext(nc) as tc, ExitStack() as ctx:
        consts = ctx.enter_context(tc.tile_pool(name="consts", bufs=1))
        xin_pool = ctx.enter_context(tc.tile_pool(name="xin", bufs=4))
        hs_pool = ctx.enter_context(tc.tile_pool(name="hs", bufs=4))
        osb_pool = ctx.enter_context(tc.tile_pool(name="osb", bufs=8))
        ht_pool = ctx.enter_context(tc.tile_pool(name="ht", bufs=6, space="PSUM"))
        ops_pool = ctx.enter_context(tc.tile_pool(name="ops", bufs=2, space="PSUM"))

        w1_sb = consts.tile([128, 2, HID], BF16_T)
        w2_sb = consts.tile([128, NHB], BF16_T)

        # PE runs at a cold clock for the first ~5us of activity; a few zero
        # matmuls start the HAM ramp while the first DMAs are in flight.
        warm_in = consts.tile([128, 512], BF16_T)
        nc.gpsimd.memset(warm_in, 0.0)
        warm_ps = ht_pool.tile([128, NWP], F32, tag="ht")
        for _ in range(N_WARM):
            nc.tensor.matmul(
                warm_ps[:, 0:N_WARM_COLS],
                warm_in[:, 0:128],
                warm_in[:, 0:N_WARM_COLS],
                start=True,
                stop=True,
            )
        if b1_nonzero:
            b1_sb = consts.tile([1, HID], BF16_T)
            nc.scalar.dma_start(out=b1_sb, in_=b1_d[:, :])
            ones_sb = consts.tile([1, NWP], BF16_T)
            nc.vector.memset(ones_sb, 1.0)

        first = True

        def emit_l2_mms(prev, v2, col):
            # L2 strip matmuls for one tile + the 2-op DVE merge into column
            # `col` of the shared pair buffer v2 [33, 2, NWP]. Four concurrent
            # column strips (PE col groups 0/32/64/96, psum partitions
            # likewise), two accumulating matmuls per strip: ~2 slots.
            b, g, hs = prev
            ops = ops_pool.tile([97, NWP], F32)
            for k in range(2):
                for s_i, part in enumerate((0, 32, 64, 96)):
                    hb = 4 * k + s_i
                    nc.tensor.matmul(
                        ops[part : part + 1, :],
                        w2_sb[:, hb : hb + 1],
                        hs[:, hb, :],
                        start=(k == 0),
                        stop=(k == 1),
                        tile_position=(0, part),
                    )
            # merge on VectorE (one PSUM operand per DVE op; partition shifts
            # must be multiples of 32): stage {64,96} down by 64 with +b2/2
            # each lane (b2 lands twice across the two lanes), then add slab
            # {0..32}. v2[:, col] lanes 0/32 hold the two partial sums.
            u = osb_pool.tile([33, NWP], F32, tag="u")
            nc.vector.tensor_scalar_add(u, ops[64:97, :], float(b2_val) * 0.5)
            nc.vector.tensor_tensor(
                v2[:, col, :], ops[0:33, :], u, mybir.AluOpType.add
            )

        def emit_l2_pair(prevA, prevB):
            # two consecutive tiles' y blocks are contiguous in DRAM (within
            # an image, and across the b/b+1 seam), so each lane row of the
            # shared v2 folds with ONE accumulating SWDGE DMA per lane onto
            # the zero-initialized 2-tile y span (same gpsimd queue -> ordered
            # read-modify-write).
            bA, gA, _ = prevA
            v2 = osb_pool.tile([33, 2, NWP], F32, tag="v2")
            emit_l2_mms(prevA, v2, 0)
            emit_l2_mms(prevB, v2, 1)
            pstep = v2.ap[0][0]
            y_flat = y_d.reshape([BPC * OH * OW])
            off = (bA * NG + gA) * NWP
            for row in (0, 32):
                src = bass.AP(
                    tensor=v2.tensor,
                    offset=v2.offset + row * pstep,
                    ap=[[pstep, 1], [NWP, 2], [1, NWP]],
                )
                nc.gpsimd.dma_start(
                    out=y_flat[off : off + 2 * NWP],
                    in_=src,
                    accum_op=mybir.AluOpType.add,
                )

        def emit_l2(prev, tail=False):
            # single-tile fallback (tail and odd flush tiles)
            b, g, hs = prev
            v2 = osb_pool.tile([33, 2, NWP], F32, tag="v2")
            emit_l2_mms(prev, v2, 0)
            pstep = v2.ap[0][0]
            if tail:
                # direct path: fold lanes on VectorE, one sync-ring y-DMA (no
                # accum-DMA SWDGE latency at the kernel tail)
                vs = osb_pool.tile([1, NWP], F32, tag="vs")
                nc.vector.tensor_scalar_add(vs, v2[32:33, 0, :], 0.0)
                osb = osb_pool.tile([1, NWP], F32)
                nc.vector.tensor_tensor(
                    osb, v2[0:1, 0, :], vs, mybir.AluOpType.add
                )
                out_src = bass.AP(
                    tensor=osb.tensor,
                    offset=osb.offset,
                    ap=[osb.ap[0], [OW, G], [1, OW]],
                )
                nc.sync.dma_start(out=y_d[b, g * G : (g + 1) * G, :], in_=out_src)
            else:
                for row in (0, 32):
                    src = bass.AP(
                        tensor=v2.tensor,
                        offset=v2.offset + row * pstep,
                        ap=[[pstep, 1], [OW, G], [1, OW]],
                    )
                    nc.gpsimd.dma_start(
                        out=y_d[b, g * G : (g + 1) * G, :],
                        in_=src,
                        accum_op=mybir.AluOpType.add,
                    )

        pend = []
        for b in range(BPC):
            for g in range(NG):
                xin = xin_pool.tile([128, 2, NWP], BF16_T)
                if first:
                    # cold-window DMA plan, by observed queue speed and hb-k
                    # consumption deadline: sync (fastest, starts ~8.7us)
                    # carries x0 both chunks + the last w1 blocks; scalar
                    # (slowest) only hb4-5; gpsimd hb0-3 in first-use order
                    # so tile-0's matmuls gate only on their own slices.
                    nc.sync.dma_start(out=xin[:, 0, :], in_=x_d[b, g, :, 0, :])
                    nc.sync.dma_start(out=xin[:, 1, :], in_=x_d[b, g, :, 1, :])
                    for lo in range(0, 512, 128):
                        nc.gpsimd.dma_start(
                            out=w1_sb[:, :, lo : lo + 128],
                            in_=w1_d[:, :, lo : lo + 128],
                        )
                    for lo in range(512, 768, 128):
                        nc.scalar.dma_start(
                            out=w1_sb[:, :, lo : lo + 128],
                            in_=w1_d[:, :, lo : lo + 128],
                        )
                    for lo in range(768, HID, 128):
                        nc.sync.dma_start(
                            out=w1_sb[:, :, lo : lo + 128],
                            in_=w1_d[:, :, lo : lo + 128],
                        )
                    nc.gpsimd.dma_start(out=w2_sb, in_=w2_d[:, :])
                else:
                    nc.sync.dma_start(out=xin, in_=x_d[b, g])
                first = False

                hs = hs_pool.tile([128, NHB, NWP], BF16_T)
                for hb in HB_ORDER:
                    ht = ht_pool.tile([128, NWP], F32)
                    if b1_nonzero:
                        nc.tensor.matmul(
                            ht[:, 0:NWP],
                            b1_sb[:, hb * 128 : (hb + 1) * 128],
                            ones_sb[:, :],
                            start=True,
                            stop=False,
                        )
                    for c in range(2):
                        nc.tensor.matmul(
                            ht[:, 0:NWP],
                            w1_sb[:, c, hb * 128 : (hb + 1) * 128],
                            xin[:, c, :],
                            start=(c == 0 and not b1_nonzero),
                            stop=(c == 1),
                        )
                    if hb in SCAL_HB:
                        nc.scalar.activation(
                            out=hs[:, hb, :], in_=ht[:, 0:NWP], func=relu
                        )
                    else:
                        nc.vector.tensor_scalar_max(
                            hs[:, hb, :], ht[:, 0:NWP], 0.0
                        )

                # batch the pipelined L2s two tiles at a time: one strip
                # transition (~280ns of PE drain-wait) per two tiles
                pend.append((b, g, hs))
                t = b * NG + g
                if t >= BPC * NG - 3:
                    # near the kernel tail: flush immediately so the final
                    # merges + output DMAs overlap the remaining compute
                    while pend:
                        p_ = pend.pop(0)
                        tl = p_[0] * NG + p_[1] >= BPC * NG - N_TAIL
                        emit_l2(p_, tail=tl)
                elif len(pend) >= 2 and t % 2 == 0:
                    emit_l2_pair(pend.pop(0), pend.pop(0))
        for p_ in pend:
            emit_l2(p_, tail=True)

    nc.finalize()
    return nc


def kernel(x, W1, b1, W2, b2):
    global LAST_RESULTS
    x = np.asarray(x, dtype=np.float32)
    W1 = np.asarray(W1, dtype=np.float32)
    b1 = np.asarray(b1, dtype=np.float32)
    W2 = np.asarray(W2, dtype=np.float32)
    b2 = np.asarray(b2, dtype=np.float32)

    xb = x.astype(BF16)
    # dense im2col phase layout (see module docstring): per (tile, partition,
    # K-chunk) 441 contiguous cols [i (7 window-rows) x j (63 cols)]
    xd = np.empty((B, NG, 128, 2, NWP), dtype=BF16)
    for p in range(128):
        kh, kwp = p // S, p % S
        for c in range(2):
            xs = xb[:, kh::S, kwp + S * c :: S][:, :OH, :OW]  # [B, 63, 63]
            xd[:, :, p, c, :] = xs.reshape(B, NG, NWP)

    # W1 row r = kh*16 + kw; chunk c, partition p=(kh*8+kwp) <- row kh*16 + 8*c + kwp
    w1p = (
        W1.reshape(KK, 2, S, HID).transpose(0, 2, 1, 3).reshape(128, 2, HID)
    ).astype(BF16)
    w2p = W2.reshape(NHB, 128).T.copy().astype(BF16)  # [p, hb] = W2[hb*128+p]
    b1p = b1.reshape(1, HID).astype(BF16)
    b1_nonzero = bool(np.any(b1 != 0.0))
    b2_val = float(b2.reshape(-1)[0])

    nc = _build_nc(b2_val, b1_nonzero)

    in_maps = []
    for c in range(NCORES):
        in_maps.append(
            {
                "x": np.ascontiguousarray(xd[c * BPC : (c + 1) * BPC]),
                "w1": w1p,
                "w2": w2p,
                "b1": b1p,
            }
        )

    LAST_RESULTS = run_bass_kernel_spmd(
        nc,
        in_maps,
        core_ids=list(range(NCORES)),
        trace=bool(int(os.environ.get("KERNEL_TRACE", "0") or "0")),
    )
    y = np.concatenate([r["y"] for r in LAST_RESULTS.results], axis=0)
    return y.astype(np.float32)

